# revision 1
# baseline (speedup 1.0000x reference)
"""GenAttentionMask packed-ragged kernel for 8 Trainium2 NeuronCores.

Semantics (matches the reference):
  for each sample i: take mask[i, :s_i, :s_i], flatten to s_i^2 elements,
  tile it num_heads times; concatenate all pieces -> 1D fp16 buffer of
  length num_heads * sum(s_i^2).

Device strategy (memory-bound, pure data movement):
  - Host packs the ragged blocks into one contiguous stream P (this is the
    sharding/layout step; ~1/17 of total traffic).
  - P is split into 8 equal ranges of Q elements, one per core: perfect
    load balance regardless of the ragged lengths (padding < 128 els/core).
  - Every core runs the SAME static program (SPMD requires one NEFF): for
    each tile of its range: DMA HBM->SBUF once, DMA SBUF->HBM num_heads
    times (optionally as ONE broadcast-AP DMA). Read 1x, write 16x == the
    roofline traffic.
  - Host assembles the final ragged concat from the per-core outputs with
    contiguous slice copies only (it knows exactly where every tile landed).
"""

import numpy as np

P_DIM = 128
MAX_COLS = 4096  # 1 MiB fp16 tiles
NCORES = 8

_NC_CACHE = {}


def _tile_sizes_cols(cpp):
    sizes = []
    left = int(cpp)
    while left > 0:
        c = min(MAX_COLS, left)
        sizes.append(c)
        left -= c
    return sizes


def _build_nc(sizes_cols, R, reps=1, broadcast=True, alt=False, bufs=4,
              nsplit=1, loop_n=0):
    """One static SPMD program: per tile, one load then R stores (or one
    broadcast store writing all R replicas; nsplit>1 splits the replica dim
    across that many broadcast DMAs).

    reps>1 unrolls the whole job back-to-back; loop_n>0 instead wraps one
    job in a Tile For_i loop (a full drain per back-edge), which measures
    isolated per-invocation time. Both are benchmarking aids.
    """
    import concourse.bacc as bacc
    import concourse.mybir as mybir
    from concourse import tile

    sizes_cols = list(sizes_cols)
    Q = P_DIM * sum(sizes_cols)
    nc = bacc.Bacc("TRN2", target_bir_lowering=False, debug=False,
                   num_devices=NCORES)
    inp = nc.dram_tensor("inp", [Q], mybir.dt.float16,
                         kind="ExternalInput").ap()
    out = nc.dram_tensor("out", [R * Q], mybir.dt.float16,
                         kind="ExternalOutput").ap()

    def body_phased(pool):
        # All DMAs on one HWDGE ring, loads emitted first: ring FIFO gives a
        # pure-read phase then a pure-write phase (no HBM R/W interleaving).
        tiles = []
        off = 0
        for c in sizes_cols:
            els = P_DIM * c
            t = pool.tile([P_DIM, c], mybir.dt.float16)
            nc.scalar.dma_start(
                t[:], inp[off:off + els].rearrange("(p c) -> p c", p=P_DIM))
            tiles.append((t, off, c, els))
            off += els
        rr = R // nsplit
        for t, off, c, els in tiles:
            for v in range(nsplit):
                r0 = v * rr
                dst = out[R * off + r0 * els:
                          R * off + (r0 + rr) * els].rearrange(
                              "(r p c) -> p r c", r=rr, p=P_DIM)
                nc.scalar.dma_start(
                    dst, t[:].unsqueeze(1).broadcast_to([P_DIM, rr, c]))

    def body(pool):
        off = 0
        for j, c in enumerate(sizes_cols):
            els = P_DIM * c
            t = pool.tile([P_DIM, c], mybir.dt.float16)
            nc.sync.dma_start(
                t[:], inp[off:off + els].rearrange("(p c) -> p c", p=P_DIM))
            if broadcast:
                rr = R // nsplit
                for v in range(nsplit):
                    r0 = v * rr
                    dst = out[R * off + r0 * els:
                              R * off + (r0 + rr) * els].rearrange(
                                  "(r p c) -> p r c", r=rr, p=P_DIM)
                    src = t[:].unsqueeze(1).broadcast_to([P_DIM, rr, c])
                    e = nc.scalar if (not alt or (j + v) % 2 == 0) \
                        else nc.sync
                    e.dma_start(dst, src)
            else:
                for r in range(R):
                    dst = out[R * off + r * els:
                              R * off + (r + 1) * els].rearrange(
                                  "(p c) -> p c", p=P_DIM)
                    e = nc.scalar if r % 2 else nc.sync
                    e.dma_start(dst, t[:])
            off += els

    b = body_phased if alt == "phase" else body
    with tile.TileContext(nc) as tc:
        with tc.tile_pool(name="sbuf", bufs=bufs) as pool:
            if loop_n:
                with tc.For_i(0, loop_n, 1):
                    b(pool)
            else:
                for _ in range(reps):
                    b(pool)
    nc.compile()
    return nc


def _get_nc(sizes_cols, R, reps=1, broadcast=True, alt=False, bufs=4,
            nsplit=1, loop_n=0):
    key = (tuple(sizes_cols), R, reps, broadcast, alt, bufs, nsplit, loop_n)
    if key not in _NC_CACHE:
        _NC_CACHE[key] = _build_nc(sizes_cols, R, reps, broadcast, alt,
                                   bufs, nsplit, loop_n)
    return _NC_CACHE[key]


def _plan(lens):
    """Pack layout: per-sample packed sizes/offsets and per-core quota."""
    s2 = lens.astype(np.int64) ** 2
    T = int(s2.sum())
    pbase = np.zeros(len(lens) + 1, np.int64)
    pbase[1:] = np.cumsum(s2)
    cpp = -(-T // (NCORES * P_DIM))  # ceil cols-per-partition per core
    sizes_cols = _tile_sizes_cols(cpp)
    Q = P_DIM * cpp
    # element offset of each tile within a core's range (+ sentinel Q)
    tprefix = np.zeros(len(sizes_cols) + 1, np.int64)
    tprefix[1:] = np.cumsum([P_DIM * c for c in sizes_cols])
    return s2, T, pbase, Q, sizes_cols, tprefix


def _pack_stream(am, lens, T, pbase, Q):
    Pstream = np.zeros(NCORES * Q, dtype=np.float16)
    for i in range(len(lens)):
        s = int(lens[i])
        Pstream[pbase[i]:pbase[i + 1]].reshape(s, s)[...] = am[i, :s, :s]
    return Pstream


def _assemble(outs, lens, s2, pbase, Q, tprefix, R):
    """outs[k] flat fp16 of length R*Q; returns the final packed concat.

    Device layout: replica r of the tile at [tprefix[j], tprefix[j+1]) lives
    at out[R*tprefix[j] + r*els_j + (g - tprefix[j])].
    """
    T = int(pbase[-1])
    F = np.empty(R * T, dtype=np.float16)
    # global cut positions: every core boundary + tile boundary
    tile_cuts = np.concatenate(
        [k * Q + tprefix[:-1] for k in range(NCORES)] + [[NCORES * Q]])
    for i in range(len(lens)):
        sz = int(s2[i])
        g0 = int(pbase[i])
        g1 = g0 + sz
        # cuts strictly inside (g0, g1)
        inner = tile_cuts[(tile_cuts > g0) & (tile_cuts < g1)]
        cuts = [g0] + [int(x) for x in inner] + [g1]
        for h in range(R):
            dst0 = R * g0 + h * sz
            for a, b in zip(cuts[:-1], cuts[1:]):
                k = a // Q
                loc = a - k * Q
                j = int(np.searchsorted(tprefix, loc, side="right")) - 1
                els_j = int(tprefix[j + 1] - tprefix[j])
                st = R * int(tprefix[j]) + h * els_j + (loc - int(tprefix[j]))
                F[dst0 + (a - g0):dst0 + (b - g0)] = outs[k][st:st + (b - a)]
    return F


def kernel(attention_mask, seq_lengths, num_heads):
    am = np.asarray(attention_mask)
    if am.dtype != np.float16:
        am = am.astype(np.float16)
    lens = np.asarray(seq_lengths).astype(np.int64)
    R = int(np.asarray(num_heads))

    s2, T, pbase, Q, sizes_cols, tprefix = _plan(lens)
    if R == 0 or T == 0:
        return np.zeros(R * T, dtype=np.float16)
    Pstream = _pack_stream(am, lens, T, pbase, Q)
    in_maps = [{"inp": Pstream[k * Q:(k + 1) * Q]} for k in range(NCORES)]

    nsplit = 2 if R % 2 == 0 else 1
    try:
        outs = _run_device(sizes_cols, R, nsplit, in_maps)
    except Exception:
        # Transient device loss (NRT_EXEC_UNIT_UNRECOVERABLE): the terminal
        # self-recovers after a pause, but only a FRESH process can
        # reconnect — the in-process jax client stays wedged. Retry in
        # subprocesses.
        outs = _run_device_subprocess(in_maps, sizes_cols, R, nsplit)
    return _assemble(outs, lens, s2, pbase, Q, tprefix, R)


def _run_device(sizes_cols, R, nsplit, in_maps):
    from concourse.bass_utils import run_bass_kernel_spmd
    nc = _get_nc(sizes_cols, R, 1, True, False, 4, nsplit)
    res = run_bass_kernel_spmd(nc, in_maps, core_ids=list(range(NCORES)))
    return [np.asarray(res.results[k]["out"]).reshape(-1)
            for k in range(NCORES)]


def _subproc_entry(tmpdir):
    """Runs inside the retry subprocess: load staged inputs, run, save."""
    import os
    meta = np.load(os.path.join(tmpdir, "meta.npy"))
    R, nsplit, ntiles = int(meta[0]), int(meta[1]), int(meta[2])
    sizes_cols = [int(x) for x in meta[3:3 + ntiles]]
    Pstream = np.load(os.path.join(tmpdir, "pstream.npy"))
    Q = Pstream.size // NCORES
    in_maps = [{"inp": Pstream[k * Q:(k + 1) * Q]} for k in range(NCORES)]
    outs = _run_device(sizes_cols, R, nsplit, in_maps)
    np.save(os.path.join(tmpdir, "outs.npy"), np.stack(outs))


def _run_device_subprocess(in_maps, sizes_cols, R, nsplit, attempts=3):
    import os
    import subprocess
    import sys
    import tempfile
    import time

    kdir = os.path.dirname(os.path.abspath(__file__))
    with tempfile.TemporaryDirectory() as td:
        meta = np.array([R, nsplit, len(sizes_cols)] + list(sizes_cols),
                        np.int64)
        np.save(os.path.join(td, "meta.npy"), meta)
        Pstream = np.concatenate([m["inp"] for m in in_maps])
        np.save(os.path.join(td, "pstream.npy"), Pstream)
        code = (f"import sys; sys.path.insert(0, {kdir!r}); "
                f"import kernel; kernel._subproc_entry({td!r})")
        err = None
        for i in range(attempts):
            time.sleep(90 if i else 10)  # let the terminal recover first
            p = subprocess.run([sys.executable, "-c", code],
                               capture_output=True, text=True,
                               timeout=1800)
            if p.returncode == 0 and os.path.exists(
                    os.path.join(td, "outs.npy")):
                stacked = np.load(os.path.join(td, "outs.npy"))
                return [stacked[k] for k in range(NCORES)]
            err = p.stderr[-2000:]
        raise RuntimeError(f"device retries exhausted: {err}")



# revision 2
# speedup vs baseline: 1.0855x; 1.0855x over previous
"""GenAttentionMask packed-ragged kernel for 8 Trainium2 NeuronCores.

Semantics (matches the reference):
  for each sample i: take mask[i, :s_i, :s_i], flatten to s_i^2 elements,
  tile it num_heads times; concatenate all pieces -> 1D fp16 buffer of
  length num_heads * sum(s_i^2).

Device strategy (memory-bound, pure data movement):
  - Host packs the ragged blocks into one contiguous stream P (this is the
    sharding/layout step; ~1/17 of total traffic).
  - P is split into 8 equal ranges of Q elements, one per core: perfect
    load balance regardless of the ragged lengths.
  - Every core runs the SAME static program (SPMD requires one NEFF),
    split into two tiles: DMA HBM->SBUF once per tile, then ONE broadcast
    DMA per tile that writes all num_heads replicas. Read 1x, write 16x ==
    the roofline traffic.
  - Tuning facts (measured via paired For_i loop timing on this part):
    * per-partition store runs must be 256B-aligned: tile cols are kept a
      multiple of 128 (cols*2B % 256 == 0). Unaligned runs cost ~10%.
    * one broadcast store per tile (replica dim intact) beats per-replica
      stores and beats splitting the replica dim across rings.
    * two tiles let tile-1's load hide under tile-0's store (loads on the
      sync ring, stores on the scalar ring); more tiles or phase-staggered
      stores bought nothing.
    With that, stores run at ~358 GB/s/core aggregate -- the HBM limit.
  - Host assembles the final ragged concat from the per-core outputs with
    contiguous slice copies only (it knows exactly where every tile landed).
"""

import numpy as np

P_DIM = 128
COL_ALIGN = 128  # tile cols multiple of this => 256B-aligned store runs
NTILES = 2
NCORES = 8

_NC_CACHE = {}


def _tile_sizes_cols(cpp):
    """Split cpp cols into NTILES near-equal tiles, each a multiple of
    COL_ALIGN (total is cpp rounded up to COL_ALIGN)."""
    units = -(-cpp // COL_ALIGN)
    n = min(NTILES, units)
    sizes = []
    for i in range(n):
        u = units // n + (1 if i < units % n else 0)
        if u:
            sizes.append(u * COL_ALIGN)
    return sizes


def _build_nc(sizes_cols, R, loop_n=0):
    """One static SPMD program: per tile, one load (sync ring) then one
    broadcast store (scalar ring) writing all R replicas. loop_n>0 wraps
    the body in a Tile For_i loop (a full drain per back-edge) -- the
    timing harness uses it to measure isolated per-invocation time."""
    import concourse.bacc as bacc
    import concourse.mybir as mybir
    from concourse import tile

    sizes_cols = list(sizes_cols)
    Q = P_DIM * sum(sizes_cols)
    nc = bacc.Bacc("TRN2", target_bir_lowering=False, debug=False,
                   num_devices=NCORES)
    inp = nc.dram_tensor("inp", [Q], mybir.dt.float16,
                         kind="ExternalInput").ap()
    out = nc.dram_tensor("out", [R * Q], mybir.dt.float16,
                         kind="ExternalOutput").ap()

    def body(pool):
        off = 0
        for c in sizes_cols:
            els = P_DIM * c
            t = pool.tile([P_DIM, c], mybir.dt.float16)
            nc.sync.dma_start(
                t[:], inp[off:off + els].rearrange("(p c) -> p c", p=P_DIM))
            dst = out[R * off:R * (off + els)].rearrange(
                "(r p c) -> p r c", r=R, p=P_DIM)
            nc.scalar.dma_start(
                dst, t[:].unsqueeze(1).broadcast_to([P_DIM, R, c]))
            off += els

    with tile.TileContext(nc) as tc:
        with tc.tile_pool(name="sbuf", bufs=max(2, len(sizes_cols))) as pool:
            if loop_n:
                with tc.For_i(0, loop_n, 1):
                    body(pool)
            else:
                body(pool)
    nc.compile()
    return nc


def _get_nc(sizes_cols, R, loop_n=0):
    key = (tuple(sizes_cols), R, loop_n)
    if key not in _NC_CACHE:
        _NC_CACHE[key] = _build_nc(sizes_cols, R, loop_n)
    return _NC_CACHE[key]


def _plan(lens):
    """Pack layout: per-sample packed sizes/offsets and per-core quota."""
    s2 = lens.astype(np.int64) ** 2
    T = int(s2.sum())
    pbase = np.zeros(len(lens) + 1, np.int64)
    pbase[1:] = np.cumsum(s2)
    cpp = -(-T // (NCORES * P_DIM))  # ceil cols-per-partition per core
    sizes_cols = _tile_sizes_cols(cpp)
    Q = P_DIM * sum(sizes_cols)
    # element offset of each tile within a core's range (+ sentinel Q)
    tprefix = np.zeros(len(sizes_cols) + 1, np.int64)
    tprefix[1:] = np.cumsum([P_DIM * c for c in sizes_cols])
    return s2, T, pbase, Q, sizes_cols, tprefix


def _pack_stream(am, lens, T, pbase, Q):
    Pstream = np.zeros(NCORES * Q, dtype=np.float16)
    for i in range(len(lens)):
        s = int(lens[i])
        Pstream[pbase[i]:pbase[i + 1]].reshape(s, s)[...] = am[i, :s, :s]
    return Pstream


def _assemble(outs, lens, s2, pbase, Q, tprefix, R):
    """outs[k] flat fp16 of length R*Q; returns the final packed concat.

    Device layout: replica r of the tile at [tprefix[j], tprefix[j+1]) lives
    at out[R*tprefix[j] + r*els_j + (g - tprefix[j])].
    """
    T = int(pbase[-1])
    F = np.empty(R * T, dtype=np.float16)
    # global cut positions: every core boundary + tile boundary
    tile_cuts = np.concatenate(
        [k * Q + tprefix[:-1] for k in range(NCORES)] + [[NCORES * Q]])
    for i in range(len(lens)):
        sz = int(s2[i])
        g0 = int(pbase[i])
        g1 = g0 + sz
        # cuts strictly inside (g0, g1)
        inner = tile_cuts[(tile_cuts > g0) & (tile_cuts < g1)]
        cuts = [g0] + [int(x) for x in inner] + [g1]
        for h in range(R):
            dst0 = R * g0 + h * sz
            for a, b in zip(cuts[:-1], cuts[1:]):
                k = a // Q
                loc = a - k * Q
                j = int(np.searchsorted(tprefix, loc, side="right")) - 1
                els_j = int(tprefix[j + 1] - tprefix[j])
                st = R * int(tprefix[j]) + h * els_j + (loc - int(tprefix[j]))
                F[dst0 + (a - g0):dst0 + (b - g0)] = outs[k][st:st + (b - a)]
    return F


def kernel(attention_mask, seq_lengths, num_heads):
    am = np.asarray(attention_mask)
    if am.dtype != np.float16:
        am = am.astype(np.float16)
    lens = np.asarray(seq_lengths).astype(np.int64)
    R = int(np.asarray(num_heads))

    s2, T, pbase, Q, sizes_cols, tprefix = _plan(lens)
    if R == 0 or T == 0:
        return np.zeros(R * T, dtype=np.float16)
    Pstream = _pack_stream(am, lens, T, pbase, Q)
    in_maps = [{"inp": Pstream[k * Q:(k + 1) * Q]} for k in range(NCORES)]

    try:
        outs = _run_device(sizes_cols, R, in_maps)
    except Exception:
        # Transient device loss (NRT_EXEC_UNIT_UNRECOVERABLE): the terminal
        # self-recovers after a pause, but only a FRESH process can
        # reconnect — the in-process jax client stays wedged. Retry in
        # subprocesses.
        outs = _run_device_subprocess(in_maps, sizes_cols, R)
    return _assemble(outs, lens, s2, pbase, Q, tprefix, R)


def _run_device(sizes_cols, R, in_maps):
    from concourse.bass_utils import run_bass_kernel_spmd
    nc = _get_nc(sizes_cols, R)
    res = run_bass_kernel_spmd(nc, in_maps, core_ids=list(range(NCORES)))
    return [np.asarray(res.results[k]["out"]).reshape(-1)
            for k in range(NCORES)]


def _subproc_entry(tmpdir):
    """Runs inside the retry subprocess: load staged inputs, run, save."""
    import os
    meta = np.load(os.path.join(tmpdir, "meta.npy"))
    R, ntiles = int(meta[0]), int(meta[1])
    sizes_cols = [int(x) for x in meta[2:2 + ntiles]]
    Pstream = np.load(os.path.join(tmpdir, "pstream.npy"))
    Q = Pstream.size // NCORES
    in_maps = [{"inp": Pstream[k * Q:(k + 1) * Q]} for k in range(NCORES)]
    outs = _run_device(sizes_cols, R, in_maps)
    np.save(os.path.join(tmpdir, "outs.npy"), np.stack(outs))


def _run_device_subprocess(in_maps, sizes_cols, R, attempts=3):
    import os
    import subprocess
    import sys
    import tempfile
    import time

    kdir = os.path.dirname(os.path.abspath(__file__))
    with tempfile.TemporaryDirectory() as td:
        meta = np.array([R, len(sizes_cols)] + list(sizes_cols), np.int64)
        np.save(os.path.join(td, "meta.npy"), meta)
        Pstream = np.concatenate([m["inp"] for m in in_maps])
        np.save(os.path.join(td, "pstream.npy"), Pstream)
        code = (f"import sys; sys.path.insert(0, {kdir!r}); "
                f"import kernel; kernel._subproc_entry({td!r})")
        err = None
        for i in range(attempts):
            time.sleep(90 if i else 10)  # let the terminal recover first
            p = subprocess.run([sys.executable, "-c", code],
                               capture_output=True, text=True,
                               timeout=1800)
            if p.returncode == 0 and os.path.exists(
                    os.path.join(td, "outs.npy")):
                stacked = np.load(os.path.join(td, "outs.npy"))
                return [stacked[k] for k in range(NCORES)]
            err = p.stderr[-2000:]
        raise RuntimeError(f"device retries exhausted: {err}")
